# revision 22
# baseline (speedup 1.0000x reference)
"""Trainium2 Bass kernel for nn_EnvironmentalAugmentations (v5).

Computes, for waveform/white_noise of shape [256, 220500] fp32:
    pink  = first-order IIR of white_noise along time:
            f[0] = w[0];  f[t] = 0.99*f[t-1] + 0.01*w[t]
    mixed = waveform + 0.05 * pink
    out   = mixed / max(max|mixed|, 1.0)     (global max over all elements)

Strategy (8 NeuronCores, data-parallel over channels, 32/core):
  * Channels processed in pairs: tile [126 x 3500] holds channel A in
    partitions 0..62, channel B in 63..125; partition p covers 3500
    consecutive samples (63 blocks/channel, a^3500 ~ 5e-16 between blocks).
  * No DRAM spill: mixed stays SBUF-resident in bf16 (16 pairs x
    7KB/partition); tolerance is 2e-2, bf16 costs ~4e-3.  DMA traffic is
    at the 85MB/core floor.
  * Two CUSTOM DVE ops (registered into concourse.dve_ops at import,
    compiled into the per-NEFF DVE table -- no firmware change):
      ENVAUG_IIR:  out = cumsum(w * p) * a^(k+1) * (0.05*b)
                   with p[k] = a^-(k+1) streamed via in1 -- the rescaled
                   prefix-scan form of the IIR, 1 elem/cycle vs the 2.1x
                   slower stock tensor_tensor_scan, fp32 in -> bf16 out
                   (kills the separate ACT prescale too).
      ENVAUG_MIX:  out = y + wav + a^(k+1)*carry ; accum_out = max(out)
                   -- fuses the carry correction (full width, not just
                   the first K columns), the mix add, and the max reduce.
  * Remaining per-pair DVE work: one stock min-reduce (for max|.| =
    max(max, -min)).  PE builds the cross-partition carry column
    (masked superdiagonal shift + t=0 injection matmuls into PSUM);
    ACT converts waveform fp32->bf16 and copies scan boundary columns.
  * Global max: gpsimd partition_all_reduce -> 4B AllReduce(max) over
    8 cores -> scale = 1/max(m,1).
  * Phase 2: rescale bf16 -> fp32 staging, alternating ACT/DVE; stores
    round-robin over the sync/scalar/gpsimd DMA queues.
"""

import numpy as np

# fp32-rounded constants, mirrored from the reference
_A = float(np.float32(0.99))
_B = float(np.float32(0.01))
_NOISE = float(np.float32(0.05))

C_FULL, T_FULL = 256, 220500
N_CORES = 8
C_PER = C_FULL // N_CORES  # 32
P_USED = 126
L = T_FULL // P_USED       # 1750  (126*1750 == 220500 exactly)


def _register_custom_ops():
    """Register the two fused DVE ops with concourse's custom-DVE table.

    Appending to dve_ops.OPS is the documented extension mechanism; the
    micro-op programs are compiled into the per-NEFF DVE table at
    compile-bir time.  Idempotent."""
    import concourse.dve_ops as dops
    from concourse.dve_spec import (
        Spec, Src0, Src1, C0, C1, C2, scan, AluOp, lower, _has_src1,
    )
    from concourse.dve_uop import DveOpSpec

    if "ENVAUG_IIR" in dops._SUB_OPCODE_FOR_NAME:
        return
    body1 = scan(AluOp.ADD, Src0 * Src1) * scan(AluOp.MULTIPLY, C1) * C2
    spec1 = Spec(body=body1)
    body2 = Src0 + Src1 + scan(AluOp.MULTIPLY, C1) * C0
    spec2 = Spec(body=body2, accum=AluOp.MAX)

    for name, spec in (("ENVAUG_IIR", spec1), ("ENVAUG_MIX", spec2)):
        opcode = dops._CUSTOM_DVE_ROW_BASE + len(dops.OPS)
        shas = {}
        for ver in ("v3", "v4"):
            ds = DveOpSpec(
                name=name, opcode=opcode, uops=lower(spec, ver=ver),
                rd1_en=_has_src1(spec),
            )
            shas[ver] = ds.sha(ver)
        op = dops.DveOp(name=name, spec=spec, subdim=False, uops_sha=shas)
        dops.OPS.append(op)
        dops._SUB_OPCODE_FOR_NAME[name] = opcode
        dops.CUSTOM_DVE_SPECS[name] = spec


def _host_consts(p_used, l):
    """Constants for the pair-stacked layout [p_used, 2*l]."""
    a64, b64 = _A, _B
    nb = p_used // 2
    lp = 2 * l
    import ml_dtypes
    k = np.arange(1, lp + 1, dtype=np.float64)
    # p[k] = a^-(k+1): the rescaled-scan stream for ENVAUG_IIR (bf16)
    prow = np.power(1.0 / a64, k).astype(ml_dtypes.bfloat16)
    prow = np.ascontiguousarray(prow[None, :])                # [1, lp]
    shift = np.zeros((p_used, p_used), dtype=np.float32)
    for p in range(p_used - 1):
        if (p + 1) % nb != 0:
            shift[p, p + 1] = 1.0
    K0 = (1.0 - b64) / (b64 * a64)
    inj = np.zeros((p_used, p_used), dtype=np.float32)
    inj[0, 0] = K0
    inj[nb, nb] = K0
    return prow, shift, inj


def build_nc(c_per=C_PER, p_used=P_USED, l=L, n_cores=N_CORES):
    """Build the Bacc module (per-core SPMD program)."""
    import concourse.mybir as mybir
    from concourse import bacc, bass_isa
    from concourse.tile import TileContext
    import concourse.dve_ops as dops

    _register_custom_ops()
    IIR = next(o for o in dops.OPS if o.name == "ENVAUG_IIR")
    MIX = next(o for o in dops.OPS if o.name == "ENVAUG_MIX")

    f32 = mybir.dt.float32
    bf16 = mybir.dt.bfloat16
    Alu = mybir.AluOpType
    AxX = mybir.AxisListType.X
    t_loc = p_used * l
    assert p_used % 2 == 0
    assert c_per % 2 == 0
    nb = p_used // 2          # blocks per channel
    lp = 2 * l                # stacked row length
    n_grp = c_per // 2        # channel pairs

    SC2 = _NOISE * _B         # 0.05*b, folded into ENVAUG_IIR's imm2

    nc = bacc.Bacc(
        "TRN2", target_bir_lowering=False, debug=False, num_devices=n_cores
    )
    wave_h = nc.dram_tensor("waveform", [c_per, t_loc], f32, kind="ExternalInput")
    noise_h = nc.dram_tensor("white_noise", [c_per, t_loc], f32, kind="ExternalInput")
    prow_h = nc.dram_tensor("prow", [1, lp], bf16, kind="ExternalInput")
    shift_h = nc.dram_tensor("shiftmat", [p_used, p_used], f32, kind="ExternalInput")
    inj_h = nc.dram_tensor("injmat", [p_used, p_used], f32, kind="ExternalInput")
    out_h = nc.dram_tensor("out", [c_per, t_loc], f32, kind="ExternalOutput")

    # [(c nb), lp] views: row c*nb+p is block p of channel c (contiguous 14KB)
    wave_r = wave_h.rearrange("c (p l) -> (c p) l", p=nb)
    noise_r = noise_h.rearrange("c (p l) -> (c p) l", p=nb)
    out_r = out_h.rearrange("c (p l) -> (c p) l", p=nb)

    with TileContext(nc) as tc:
        with (
            tc.tile_pool(name="const", bufs=1) as constp,
            tc.tile_pool(name="dram", bufs=1, space="DRAM") as dramp,
        ):
            # bf16 p-stream (8-bit exponent covers a^-3500 ~ 1.9e15; the
            # 0.4% mantissa error is far below the bf16 noise of mixed)
            prow_bf = constp.tile([1, lp], bf16, tag="prowbf")
            nc.scalar.dma_start(out=prow_bf[:], in_=prow_h[:, :])
            shift_t = constp.tile([p_used, p_used], f32, tag="shift")
            nc.scalar.dma_start(out=shift_t[:], in_=shift_h[:, :])
            inj_t = constp.tile([p_used, p_used], f32, tag="injmat")
            nc.scalar.dma_start(out=inj_t[:], in_=inj_h[:, :])
            ptile = constp.tile([p_used, lp], bf16, tag="ptile")
            nc.gpsimd.partition_broadcast(
                ptile[:], prow_bf[0:1, :], channels=p_used
            )
            maxcols = constp.tile([p_used, n_grp], f32, tag="maxcols")
            mincols = constp.tile([p_used, n_grp], f32, tag="mincols")

            with (
                tc.tile_pool(name="resp", bufs=1) as resp,
                tc.tile_pool(name="iow", bufs=3) as iow,
                tc.tile_pool(name="iov", bufs=3) as iov,
                tc.tile_pool(name="colp", bufs=4) as colp,
                tc.tile_pool(name="cps", bufs=4, space="PSUM") as cpsp,
            ):
                mix_tiles = [
                    resp.tile([p_used, lp], bf16, tag=f"mix{g}",
                              name=f"mix{g}")
                    for g in range(n_grp)
                ]
                w_tiles, wav_tiles, wavbf_tiles, carry_ps = {}, {}, {}, {}

                def emit_loads(g):
                    rows = slice(g * p_used, (g + 1) * p_used)
                    w = iow.tile([p_used, lp], f32, tag="w")
                    nc.sync.dma_start(out=w[:], in_=noise_r[rows, :])
                    wv = iov.tile([p_used, lp], f32, tag="wav")
                    nc.scalar.dma_start(out=wv[:], in_=wave_r[rows, :])
                    w_tiles[g], wav_tiles[g] = w, wv

                DEPTH = 2
                for g in range(min(DEPTH, n_grp)):
                    emit_loads(g)

                for g in range(n_grp + 2):
                    if g < n_grp:
                        if g + DEPTH < n_grp:
                            emit_loads(g + DEPTH)
                        mix = mix_tiles[g]
                        w_t = w_tiles.pop(g)
                        wav_t = wav_tiles.pop(g)
                        # DVE: fused rescaled IIR scan, fp32 in -> bf16 out
                        nc.vector._custom_dve(
                            IIR, out=mix[:], in0=w_t[:], in1=ptile[:],
                            s1=_A, imm2=SC2,
                        )
                        # waveform stays fp32; MIX reads it directly
                        wavbf_tiles[g] = wav_t
                        # ACT: boundary columns bf16 -> fp32 for the PE carry
                        col_last = colp.tile([p_used, 1], f32, tag="cl")
                        nc.scalar.copy(col_last[:], mix[:, lp - 1 : lp])
                        col_first = colp.tile([p_used, 1], f32, tag="cf")
                        nc.scalar.copy(col_first[:], mix[:, 0:1])
                        # PE: carry column in PSUM
                        cp = cpsp.tile([p_used, 1], f32, tag="carry")
                        nc.tensor.matmul(
                            cp[:], shift_t[:], col_last[:], start=True, stop=False
                        )
                        nc.tensor.matmul(
                            cp[:], inj_t[:], col_first[:], start=False, stop=True
                        )
                        carry_ps[g] = cp
                    if 1 <= g <= n_grp:
                        gp = g - 1
                        # DVE: fused carry-correction + mix + max accum
                        nc.vector._custom_dve(
                            MIX, out=mix_tiles[gp][:], in0=mix_tiles[gp][:],
                            in1=wavbf_tiles.pop(gp)[:],
                            s0=carry_ps.pop(gp)[:, 0:1], s1=_A,
                            accum_out=maxcols[:, gp : gp + 1],
                        )
                    if g >= 2:
                        gq = g - 2
                        # DVE: min reduce (for max|.| = max(max, -min))
                        nc.vector.tensor_reduce(
                            mincols[:, gq : gq + 1], mix_tiles[gq][:], AxX,
                            Alu.min,
                        )

                # ---- global max + scale ----
                allmax = constp.tile([p_used, 1], f32, tag="allmax")
                nc.vector.tensor_reduce(
                    allmax[:], maxcols[:, 0:n_grp], AxX, Alu.max
                )
                allmin = constp.tile([p_used, 1], f32, tag="allmin")
                nc.vector.tensor_reduce(
                    allmin[:], mincols[:, 0:n_grp], AxX, Alu.min
                )
                # max|.| = max(allmax, -allmin)
                nc.vector.tensor_scalar_mul(allmin[:], allmin[:], -1.0)
                nc.vector.tensor_tensor(
                    allmax[:], allmax[:], allmin[:], Alu.max
                )
                gmax = constp.tile([p_used, 1], f32, tag="gmax")
                nc.gpsimd.partition_all_reduce(
                    gmax[:], allmax[:], channels=p_used,
                    reduce_op=bass_isa.ReduceOp.max,
                )
                sc_b = constp.tile([p_used, 1], f32, tag="scb")
                if n_cores > 1:
                    cc_in = dramp.tile([1, 1], f32, tag="ccin")
                    cc_out = dramp.tile([1, 1], f32, tag="ccout")
                    nc.sync.dma_start(out=cc_in[:], in_=gmax[0:1, 0:1])
                    nc.gpsimd.collective_compute(
                        "AllReduce",
                        Alu.max,
                        replica_groups=[list(range(n_cores))],
                        ins=[cc_in[:]],
                        outs=[cc_out[:]],
                    )
                    sc_small = constp.tile([1, 1], f32, tag="scsmall")
                    nc.sync.dma_start(out=sc_small[:], in_=cc_out[:])
                    nc.gpsimd.partition_broadcast(
                        sc_b[:], sc_small[0:1, 0:1], channels=p_used
                    )
                else:
                    nc.vector.tensor_copy(sc_b[:], gmax[:])
                # scale = 1 / max(gmax, 1.0)
                nc.vector.tensor_scalar_max(sc_b[:], sc_b[:], 1.0)
                inv_t = constp.tile([p_used, 1], f32, tag="inv")
                nc.vector.reciprocal(inv_t[:], sc_b[:])

                # ---- phase 2: rescale bf16 -> fp32 on DVE (runs at 2x,
                # ~1.7us/pair, and keeps the three DMA-trigger engines
                # free); stores round-robin on 3 queues.  Staging reuses
                # the idle load pools (5 bufs).
                dmas = [nc.sync, nc.scalar, nc.gpsimd]
                for g in range(n_grp):
                    rows = slice(g * p_used, (g + 1) * p_used)
                    if g % 5 < 3:
                        st = iow.tile([p_used, lp], f32, tag="w")
                    else:
                        st = iov.tile([p_used, lp], f32, tag="wav")
                    nc.vector.tensor_scalar_mul(
                        st[:], mix_tiles[g][:], inv_t[:, 0:1]
                    )
                    dmas[g % 3].dma_start(out=out_r[rows, :], in_=st[:])

    nc.compile()
    return nc


_CACHE = {}
LAST_RESULTS = None


def run(waveform, white_noise, c_per=C_PER, p_used=P_USED, l=L, n_cores=N_CORES,
        **spmd_kwargs):
    """Shard inputs over n_cores, run the SPMD bass kernel, gather output."""
    global LAST_RESULTS
    from concourse.bass_utils import run_bass_kernel_spmd

    key = (c_per, p_used, l, n_cores)
    if key not in _CACHE:
        _CACHE[key] = build_nc(c_per, p_used, l, n_cores)
    nc = _CACHE[key]

    prow, shift, inj = _host_consts(p_used, l)
    waveform = np.ascontiguousarray(waveform, dtype=np.float32)
    white_noise = np.ascontiguousarray(white_noise, dtype=np.float32)

    in_maps = []
    for i in range(n_cores):
        sl = slice(i * c_per, (i + 1) * c_per)
        in_maps.append({
            "waveform": np.ascontiguousarray(waveform[sl]),
            "white_noise": np.ascontiguousarray(white_noise[sl]),
            "prow": prow,
            "shiftmat": shift,
            "injmat": inj,
        })

    res = run_bass_kernel_spmd(nc, in_maps, core_ids=list(range(n_cores)),
                               **spmd_kwargs)
    LAST_RESULTS = res
    return np.concatenate([r["out"] for r in res.results], axis=0)


def kernel(waveform, white_noise):
    return run(waveform, white_noise)
